# revision 58
# baseline (speedup 1.0000x reference)
"""Trainium2 Bass kernel for nn_ContinuousAttention (B=32, L=2999, D=512, NB=16).

Math (per example b):
    u      = W_enc @ q[b]                      (D,)
    s[l]   = keys[b,l,:] . u / sqrt(D)         (L,)   raw scores
    w[l]   = exp(s[l])                          -- no max-subtraction needed:
                                                  s ~ N(0,1), |s| < ~6, exp safe
    Z      = sum w;  S1 = sum w*pos;  S2 = sum w*pos^2
    mu     = S1/Z;  var = clip(S2/Z - mu^2, 1e-7)
    tv_j   = var + basis_sigma_j^2
    r_j    = (1/sqrt(2pi)) / sqrt(tv_j) * exp(-0.5 (mu - mu_j)^2 / tv_j)
    BmatT  = G^T @ values[b]                   (NB, D)  [= (values^T G)^T]
    c[b]   = r . BmatT                         (D,)

Sharding: data-parallel over batch, 4 examples per core x 8 cores.

v18 design (evolved across many traced iterations):
  - The per-core DMA engine pool caps at ~392 GB/s aggregate no matter how
    many queues are used, so stream time is set by BYTES: keys ship fp8
    e4m3, values fp8 e3m4 (12.3 MB/core), W/q fp8 e4m3 (W pre-scaled by 16
    against e4m3's subnormal floor; the 1/16 folds into the exp scale).
    Measured end-to-end rel-err ~1.2e-2 vs the 2e-2 gate.
  - Scores run as DoubleRow fp8 matmuls (2 k-subtiles per pass, measured
    215 ns/512 cols warm = the true 2x rate) into a QUADRANT PSUM layout:
    l is split into 4 blocks, each written to its own PE column group
    (out partitions 32q..32q+3).  The [100, 384] tile is one PSUM bank
    (not three), matmuls to different col groups can overlap in the
    array, and every downstream row op (exp, S1/S2) runs on ~384 columns
    instead of ~1536 -- the engines here are free-length-bound at
    ~1 ns/col, so this quarters the softmax-stats cost.
  - lhsT carries all 4 u columns: out rows are (q_b' . k_b) -- row b real,
    rest junk at zero PE cost (engine APs can only start at partitions
    0/32/64/96 anyway).  Junk rows and the 9 zero-padded key columns are
    excluded by the stats gather: a [100,4] selection matmul sums quadrant
    partials and masks junk, a diagonal mask picks example b, and Z gets
    an exact -9 for the exp(0)=1 pad columns.
  - Score PSUM banks are zeroed once up front (junk partitions otherwise
    hold boot garbage that exp() could blow up to inf, which would poison
    the 0*inf products in the gather matmul).
  - bmat splits value subtiles even/odd into TWO psum banks at column
    groups 0 and 64, so consecutive matmuls target disjoint array column
    groups and overlap; the combine uses a [112,1] replicated-r column
    (zeros over the unused 48:64 rows; bmT is memset once so 0*junk can't
    make NaN).
  - G is tiny but precision-critical (bf16-single G costs 1.7e-2 alone,
    and the 48-term combine cancels heavily -- bf16 bmT/rT cost 3.5e-2,
    so those stay fp32): G ships as THREE e3m4 levels, each the scaled
    residual of the last (1024*32^i, exact powers of two folded into r).
  - Stream order kt0,kt1,v0,kt2,kt3,v1,v2,v3 on the single sync HWDGE
    ring: kt3 lands ~3/4 through so b3's stats chain clears while values
    still stream; the tail is the last value piece's bmat + combine.
  (Paths that fault or underperform on this HW, tried and reverted:
  float32r matmuls, fp16 anything, tensor_tensor_reduce, SWDGE cast-DMAs,
  gpsimd scalar_tensor_tensor/tensor_reduce-X (no Pool support), engine
  APs starting off 0/32/64/96, DoubleRow with <16B weight step,
  tensor_scalar-with-accum (slow path), >1-bank matmul outputs.)
"""

import numpy as np
import ml_dtypes
from contextlib import ExitStack

import concourse.bass as bass
import concourse.bacc as bacc
import concourse.tile as tile
from concourse import mybir
from concourse.bass_utils import run_bass_kernel_spmd

F32 = mybir.dt.float32
BF16 = mybir.dt.bfloat16
E4 = mybir.dt.float8e4      # ml_dtypes.float8_e4m3
E3 = mybir.dt.float8e3      # ml_dtypes.float8_e3m4
AF = mybir.ActivationFunctionType
ALU = mybir.AluOpType
DROW = mybir.MatmulPerfMode.DoubleRow

B, L, D, NB = 32, 2999, 512, 16
NCORES = 8
PER = B // NCORES              # 4 examples per core
NT = 24                        # value-stream subtiles of 128 rows
HALF_A_ROWS = 1536             # subtiles 0..11: rows [0,1536), 12 rows/partition
HALF_B_MAIN = 1408             # subtiles 12..22: rows [1536,2944), 11 rows/partition
TAIL0 = HALF_A_ROWS + HALF_B_MAIN   # 2944
NTAIL = L - TAIL0              # 55 tail rows -> partitions 0..54 of subtile 23
LB = L - HALF_A_ROWS           # 1463
LBP = 1472                     # LB padded to 4*368 (zero key columns)
NPAD = LBP - LB                # 9 pad columns -> Z overcounts by exactly 9
BLKA, BLKB = 384, 368          # quadrant widths of the two kt halves
INV_SQRT_D = float(1.0 / np.sqrt(float(D)))
INV_SQRT_2PI = float(1.0 / np.sqrt(2.0 * np.pi))
NLVL = 3                       # fp8 G levels
GS0, GLS = 1024.0, 32.0        # G level scales: S_i = GS0 * GLS**i
NW = 64 + NLVL * NB            # 112: combine column height (48 | 16 zero | 48)


def _rowmap(p, t):
    """Value-stream: global row held at (partition p, subtile t), -1 = pad."""
    if t < 12:
        return 12 * p + t
    if t < 23:
        return HALF_A_ROWS + 11 * p + (t - 12)
    return TAIL0 + p if p < NTAIL else -1


def _build_bass():
    # Bacc (not raw Bass): its compile pipeline splits multi-wait sync infos
    # into event semaphores, which the TRN2 BIR verifier requires for the
    # Tile kernel-tail drain.
    nc = bacc.Bacc(None, target_bir_lowering=False)
    kta_t = nc.declare_dram_parameter(
        "ktpa", [PER, 128, 4 * HALF_A_ROWS], E4, isOutput=False
    )
    ktb_t = nc.declare_dram_parameter("ktpb", [PER, 128, 4 * LBP], E4, isOutput=False)
    vp_t = nc.declare_dram_parameter("vp", [PER, 128, NT * D], E3, isOutput=False)
    wt_t = nc.declare_dram_parameter("wt", [128, 4, D], E4, isOutput=False)
    qt_t = nc.declare_dram_parameter("qt", [128, 4, PER], E4, isOutput=False)
    # G as NLVL scaled-residual fp8 levels
    g_t = nc.declare_dram_parameter("gp", [128, NT, NLVL, NB], E3, isOutput=False)
    # bf16 pos tables in the quadrant layout: row 32q+r holds block q;
    # cols [0,BLKA) = pos of kt-half A, [BLKA,BLKA+BLKB) = half B,
    # then the same again for pos^2 (pad positions are 0)
    posb_t = nc.declare_dram_parameter(
        "posb", [100, 2 * (BLKA + BLKB)], BF16, isOutput=False
    )
    # f32 misc: [0:4,0:16] bmu, [0:4,16:32] bsig^2, [0:16,32:48] identity16,
    # [0:4,48:60] I4 replicated x3 (diag mask), [0:100,64:68] quadrant
    # selection matrix sel[p,b] = (p%32==b)
    misc_t = nc.declare_dram_parameter("misc", [100, 80], F32, isOutput=False)
    out_t = nc.declare_dram_parameter("out", [PER, D], F32, isOutput=True)

    with ExitStack() as ctx:
        tc = ctx.enter_context(tile.TileContext(nc))
        const = ctx.enter_context(tc.tile_pool(name="const", bufs=1))
        kpa = ctx.enter_context(tc.tile_pool(name="kpa", bufs=4))
        kpb = ctx.enter_context(tc.tile_pool(name="kpb", bufs=4))
        vpool = ctx.enter_context(tc.tile_pool(name="vpool", bufs=4))
        wpool = ctx.enter_context(tc.tile_pool(name="wpool", bufs=4))
        wscp = ctx.enter_context(tc.tile_pool(name="wscp", bufs=2))
        scps = ctx.enter_context(tc.tile_pool(name="scps", bufs=4, space="PSUM"))
        pbm = ctx.enter_context(tc.tile_pool(name="pbm", bufs=4, space="PSUM"))

        # ---- constants (scalar=ACT HWDGE ring; the sync ring is keys/values
        # only).  qt+wt first -- they gate the U prologue on the PE. ----
        qt_sb = const.tile([128, 4, PER], E4, tag="qt")
        nc.scalar.dma_start(out=qt_sb, in_=qt_t[:, :, :])
        wt_sb = const.tile([128, 4, D], E4, tag="wt")
        nc.scalar.dma_start(out=wt_sb, in_=wt_t[:, :, :])
        G_sb = const.tile([128, NT, NLVL, NB], E3, tag="G")
        nc.scalar.dma_start(out=G_sb, in_=g_t[:, :, :, :])
        posb_sb = const.tile([100, 2 * (BLKA + BLKB)], BF16, tag="posb")
        nc.scalar.dma_start(out=posb_sb, in_=posb_t[:, :])
        misc_sb = const.tile([100, 80], F32, tag="misc")
        nc.scalar.dma_start(out=misc_sb, in_=misc_t[:, :])
        bmu_sb = misc_sb[0:PER, 0:16]
        sig2_sb = misc_sb[0:PER, 16:32]
        I_sb = misc_sb[0:16, 32:48]
        I4rep_sb = misc_sb[0:PER, 48 : 48 + 3 * PER]
        sel_sb = misc_sb[0:100, 64:68]

        # zero column block for psum-clearing matmuls
        zc_sb = const.tile([128, 100], E4, tag="zc")
        nc.vector.memset(zc_sb, 0.0)

        # ---- prologue: U[p, dm, b] = 16*u_b[128*dm+p] (d on partitions) ----
        # free dim padded to 16 so DoubleRow LDWEIGHTS sees step%16==0
        U_sb = const.tile([128, 4, 16], E4, tag="U")
        # zero the whole tile: cols PER..15 are only step%16 padding, but
        # boot garbage there can be NaN-pattern bytes that the DoubleRow
        # weight load may sweep into junk psum rows (seen once on HW as a
        # timing-dependent NaN output via exp(NaN) -> 0*NaN in the gather)
        nc.vector.memset(U_sb, 0.0)
        for dm in range(4):
            up = pbm.tile([128, PER], F32, tag="pbm", name=f"up{dm}")
            for et in range(4):
                nc.tensor.matmul(
                    up,
                    lhsT=wt_sb[:, et, dm * 128 : (dm + 1) * 128],
                    rhs=qt_sb[:, et, :],
                    start=(et == 0),
                    stop=(et == 3),
                )
            nc.vector.tensor_copy(out=U_sb[:, dm, :PER], in_=up)

        # ---- main stream state ----
        # statsA[p, s, b, h]: engine-accumulated partials -- s=0 Z (exp
        # accum), s=1 S1, s=2 S2 (DVE stt accums); p = 32q+row junk/real.
        statsA = const.tile([100, 3, PER, 2], F32, tag="statsA")
        bmT_sb = [
            const.tile([NW, D], F32, tag=f"bmT{b}", name=f"bmT{b}")
            for b in range(PER)
        ]
        for b in range(PER):
            nc.vector.memset(bmT_sb[b], 0.0)  # rows 48:64 never written
        rT6_sb = const.tile([NW, PER], F32, tag="rT6")
        c_sb = const.tile([1, PER, D], F32, tag="c_sb")
        k_tiles = {}
        v_tiles = {}
        bm_tiles = {}

        def load_kt(b, ring, slices=1):
            # two half-tiles (l < 1536 and l >= 1536), flat 2-D DMAs
            # (6 KB contiguous per partition); kt0 sliced for early scores
            ta = kpa.tile([128, 4, HALF_A_ROWS], E4, tag="kta")
            tb = kpb.tile([128, 4, LBP], E4, tag="ktb")
            if slices == 1:
                ring.dma_start(out=ta.rearrange("p t l -> p (t l)"), in_=kta_t[b])
            else:
                sa = kta_t[b].rearrange("p (t l) -> p t l", l=HALF_A_ROWS)
                for i in range(slices):
                    a0 = i * HALF_A_ROWS // slices
                    a1 = (i + 1) * HALF_A_ROWS // slices
                    ring.dma_start(out=ta[:, :, a0:a1], in_=sa[:, :, a0:a1])
            ring.dma_start(out=tb.rearrange("p t l -> p (t l)"), in_=ktb_t[b])
            k_tiles[b] = (ta, tb)

        def load_v(b, ring, pieces=(NT,)):
            # flat 2-D descriptors (contiguous multi-KB runs per partition;
            # the 3-D [p, s, d] pattern emits 512 B runs and streams slower);
            # pieces alternate between the two HWDGE rings
            tv = vpool.tile([128, NT, D], E3, tag="vtile")
            tvf = tv.rearrange("p s d -> p (s d)")
            s0 = 0
            for n in pieces:
                ring.dma_start(
                    out=tvf[:, s0 * D : (s0 + n) * D],
                    in_=vp_t[b, :, s0 * D : (s0 + n) * D],
                )
                s0 += n
            v_tiles[b] = tv

        def scores_ex(b):
            # Per kt half: 4 quadrant accumulation groups, each 2 DoubleRow
            # matmuls (dt pairs) writing out partitions 32q..32q+3 -- one
            # PSUM bank total, and consecutive quadrants hit different PE
            # column groups so they can overlap.  exp of all 100 rows
            # straight out of PSUM (junk rows exp(0)=1), then one S1 + one
            # S2 stt over [100, BLK] -- a quarter of the flat-row length.
            kta, ktb = k_tiles.pop(b)
            for h, (kt, blk) in enumerate(((kta, BLKA), (ktb, BLKB))):
                sc_ps = scps.tile([100, blk], F32, tag="scps", name=f"sc{b}_{h}")
                wh = wpool.tile([100, blk], BF16, tag="w4", name=f"w{b}_{h}")
                # zero the whole tile first: quadrant matmuls only write
                # partitions 32q..32q+3 and exp() reads all 100 rows
                nc.tensor.matmul(
                    sc_ps, lhsT=zc_sb[:, 0:100], rhs=wt_sb[:, 0, 0:blk],
                    start=True, stop=True, skip_group_check=True,
                )
                for q in range(4):
                    for dt in range(4):
                        nc.tensor.matmul(
                            sc_ps[32 * q : 32 * q + PER, :],
                            lhsT=U_sb[:, dt, 0:PER],
                            rhs=kt[:, dt, q * blk : (q + 1) * blk],
                            start=(dt == 0),
                            stop=(dt == 3),
                            tile_position=(0, 32 * q),
                        )
                nc.scalar.activation(
                    out=wh,
                    in_=sc_ps,
                    func=AF.Exp,
                    scale=INV_SQRT_D / 16.0,
                    accum_out=statsA[:, 0, b, h : h + 1],
                )
                o0 = 0 if h == 0 else BLKA
                p2 = BLKA + BLKB
                w1 = wscp.tile([100, blk], BF16, tag="wsc1", name=f"w1_{b}_{h}")
                nc.vector.scalar_tensor_tensor(
                    out=w1,
                    in0=wh,
                    scalar=1.0,
                    in1=posb_sb[:, o0 : o0 + blk],
                    op0=ALU.mult,
                    op1=ALU.mult,
                    accum_out=statsA[:, 1, b, h : h + 1],
                )
                w2 = wscp.tile([100, blk], BF16, tag="wsc2", name=f"w2_{b}_{h}")
                nc.vector.scalar_tensor_tensor(
                    out=w2,
                    in0=w1,
                    scalar=1.0,
                    in1=posb_sb[:, o0 : o0 + blk],
                    op0=ALU.mult,
                    op1=ALU.mult,
                    accum_out=statsA[:, 2, b, h : h + 1],
                )

        def bmat_ex(b, lo=0, hi=NT):
            # even subtiles accumulate into one bank at column group 0,
            # odd subtiles into another at column group 64: consecutive
            # matmuls target disjoint PE column groups and overlap.
            if lo == 0:
                bmo_full = pbm.tile([NW, D], F32, tag="pbm", name=f"bmo{b}")
                bm_tiles[b] = (
                    pbm.tile([NLVL * NB, D], F32, tag="pbm", name=f"bme{b}"),
                    bmo_full[64:NW, :],  # col group 64: overlaps with group 0
                )
            bm_e, bm_o = bm_tiles[b]
            vt = v_tiles[b]
            for t in range(lo, hi):
                nc.tensor.matmul(
                    bm_e if t % 2 == 0 else bm_o,
                    lhsT=G_sb[:, t, :, :],
                    rhs=vt[:, t, :],
                    start=(t < 2),
                    stop=(t >= NT - 2),
                )
            if hi == NT:
                del v_tiles[b]
                if b % 2 == 0:
                    nc.vector.tensor_copy(out=bmT_sb[b][0 : NLVL * NB, :], in_=bm_e)
                    nc.vector.tensor_copy(out=bmT_sb[b][64:NW, :], in_=bm_o)
                else:
                    nc.scalar.copy(bmT_sb[b][0 : NLVL * NB, :], bm_e)
                    nc.scalar.copy(bmT_sb[b][64:NW, :], bm_o)

        def rchain():
            # stats gather: reduce halves -> [100, 3*PER]; one selection
            # matmul sums quadrant partials and masks junk rows; diag mask
            # picks example b; Z gets the exact -NPAD pad correction.
            red = const.tile([100, 3 * PER], F32, tag="red")
            nc.vector.tensor_reduce(
                out=red.rearrange("p (s b) -> p s b", s=3),
                in_=statsA,
                axis=mybir.AxisListType.X,
                op=ALU.add,
            )
            stG = scps.tile([PER, 3 * PER], F32, tag="scps", name="stG")
            nc.tensor.matmul(stG, lhsT=sel_sb, rhs=red, start=True, stop=True)

            rs = const.tile([PER, 96], F32, tag="rsc")
            dg = rs[:, 0:12].rearrange("p (s b) -> p s b", s=3)
            nc.vector.tensor_mul(
                dg,
                stG.rearrange("p (s b) -> p s b", s=3),
                I4rep_sb.rearrange("p (s b) -> p s b", s=3),
            )
            st = rs[:, 12:15]
            nc.vector.tensor_reduce(
                out=st, in_=dg, axis=mybir.AxisListType.X, op=ALU.add
            )
            Z_sb = rs[:, 15:16]
            nc.vector.tensor_scalar(
                out=Z_sb, in0=st[:, 0:1], scalar1=-float(NPAD), scalar2=None,
                op0=ALU.add,
            )

            rZ = rs[:, 27:28]
            nc.vector.reciprocal(rZ, Z_sb)
            me = rs[:, 28:30]  # [mu, e2]
            nc.vector.tensor_scalar(
                out=me, in0=st[:, 1:3], scalar1=rZ, scalar2=None, op0=ALU.mult
            )
            mu = me[:, 0:1]
            mu2 = rs[:, 30:31]
            nc.vector.tensor_mul(mu2, mu, mu)
            var = rs[:, 31:32]
            nc.vector.tensor_sub(var, me[:, 1:2], mu2)
            nc.vector.tensor_scalar_max(var, var, 1e-7)

            tv = rs[:, 32:48]
            nc.vector.tensor_scalar(
                out=tv, in0=sig2_sb, scalar1=var, scalar2=None, op0=ALU.add
            )
            dmu = rs[:, 48:64]
            nc.vector.tensor_scalar(
                out=dmu, in0=bmu_sb, scalar1=mu, scalar2=None, op0=ALU.subtract
            )
            dmu2 = rs[:, 64:80]
            nc.vector.tensor_mul(dmu2, dmu, dmu)
            rtv = rs[:, 80:96]
            nc.vector.reciprocal(rtv, tv)
            arg = rs[:, 48:64]  # overwrite dmu (consumed)
            nc.vector.tensor_mul(arg, dmu2, rtv)
            eterm = rs[:, 64:80]  # overwrite dmu2 (consumed)
            nc.scalar.activation(out=eterm, in_=arg, func=AF.Exp, scale=-0.5)
            srtv = rs[:, 32:48]  # overwrite tv (consumed)
            nc.scalar.activation(out=srtv, in_=rtv, func=AF.Sqrt)
            r_sb = rs[:, 0:16]  # overwrite gather scratch (consumed)
            nc.vector.scalar_tensor_tensor(
                out=r_sb,
                in0=srtv,
                scalar=INV_SQRT_2PI,
                in1=eterm,
                op0=ALU.mult,
                op1=ALU.mult,
            )

            # r replicated over levels for both column groups, with the
            # level scales folded in; rows 48:64 zero
            r6_sb = const.tile([PER, NW], F32, tag="r6")
            nc.vector.memset(r6_sb[:, NLVL * NB : 64], 0.0)
            for i in range(NLVL):
                for half in (0, 64):
                    nc.scalar.mul(
                        r6_sb[:, half + i * NB : half + (i + 1) * NB],
                        r_sb,
                        1.0 / (GS0 * GLS**i),
                    )
            rT_ps = scps.tile([NW, PER], F32, tag="scps", name="rT_ps")
            nc.tensor.matmul(
                rT_ps, lhsT=r6_sb, rhs=I_sb[:PER, :PER], start=True, stop=True
            )
            nc.vector.tensor_copy(out=rT6_sb, in_=rT_ps)

        c64_sb = const.tile([65, D], F32, tag="c64")

        def combine(b):
            # c[b] = r6[b] . bmT[112]  (zero r rows kill the unused 48:64).
            # Odd examples land at PSUM partition 64 (PE col group 2) so
            # consecutive f32 combine matmuls overlap in the array.
            if b % 2 == 0:
                c_ps = scps.tile([1, D], F32, tag="scps", name=f"c_ps{b}")
                nc.tensor.matmul(
                    c_ps, lhsT=rT6_sb[:, b : b + 1], rhs=bmT_sb[b],
                    start=True, stop=True,
                )
                nc.vector.tensor_copy(out=c_sb[0:1, b, :], in_=c_ps)
                nc.sync.dma_start(out=out_t[b : b + 1, :], in_=c_sb[0:1, b, :])
            else:
                c_ps = scps.tile([65, D], F32, tag="scps", name=f"c_ps{b}")
                nc.tensor.matmul(
                    c_ps[64:65, :], lhsT=rT6_sb[:, b : b + 1], rhs=bmT_sb[b],
                    start=True, stop=True, tile_position=(0, 64),
                )
                nc.scalar.copy(c64_sb[64:65, :], c_ps[64:65, :])
                nc.scalar.dma_start(out=out_t[b : b + 1, :], in_=c64_sb[64:65, :])

        # ---- stream schedule ----
        # Single sync HWDGE ring (the engine pool is the BW cap; one queue
        # keeps ordering deterministic).  kt3 lands ~3/4 through the stream
        # so b3's stats chain clears while values still arrive.
        load_kt(0, nc.sync, slices=2)
        load_kt(1, nc.sync)
        scores_ex(0)
        load_kt(2, nc.sync)
        scores_ex(1)
        load_kt(3, nc.sync)
        scores_ex(2)
        load_v(0, nc.sync)
        scores_ex(3)
        load_v(1, nc.sync)
        bmat_ex(0)
        rchain()
        load_v(2, nc.sync, pieces=(12, 12))
        bmat_ex(1)
        load_v(3, nc.sync, pieces=(12, 10, 2))
        bmat_ex(2)
        bmat_ex(3)
        combine(0)
        combine(1)
        combine(2)
        combine(3)

    nc.finalize()
    return nc


_CACHE = {}


def _get_nc():
    if "nc" not in _CACHE:
        _CACHE["nc"] = _build_bass()
    return _CACHE["nc"]


def _pack_vstream(x):
    """(PER, L, D) f32 -> (PER, 128, NT*D) e3m4 in the p-major block layout."""
    out = np.zeros((PER, 128, NT * D), dtype=ml_dtypes.float8_e3m4)
    x8 = x.astype(ml_dtypes.float8_e3m4)
    for b in range(PER):
        blk = out[b].reshape(128, NT, D)
        blk[:, :12] = x8[b, :HALF_A_ROWS].reshape(128, 12, D)
        blk[:, 12:23] = x8[b, HALF_A_ROWS:TAIL0].reshape(128, 11, D)
        blk[:NTAIL, 23] = x8[b, TAIL0:]
    return out


def _pack_ktstream(x):
    """(PER, L, D) f32 -> two transposed e4m3 halves, each [b, p, dt, l] =
    x[b, l_half, 128*dt + p]; half B zero-padded to 4*368 l columns."""
    xt = x.reshape(PER, L, 4, 128).transpose(0, 3, 2, 1)  # (PER, 128, 4, L)
    xt = xt.astype(ml_dtypes.float8_e4m3)
    a = np.ascontiguousarray(xt[:, :, :, :HALF_A_ROWS]).reshape(PER, 128, -1)
    bpad = np.zeros((PER, 128, 4, LBP), dtype=ml_dtypes.float8_e4m3)
    bpad[:, :, :, :LB] = xt[:, :, :, HALF_A_ROWS:]
    return a, bpad.reshape(PER, 128, -1)


def make_in_maps(query, keys, values, W_enc, G, basis_mu, basis_sigma):
    query = np.asarray(query, dtype=np.float32)
    keys = np.asarray(keys, dtype=np.float32)
    values = np.asarray(values, dtype=np.float32)
    W_enc = np.asarray(W_enc, dtype=np.float32)
    G = np.asarray(G, dtype=np.float32)
    basis_mu = np.asarray(basis_mu, dtype=np.float32).reshape(1, NB)
    basis_sigma = np.asarray(basis_sigma, dtype=np.float32).reshape(1, NB)

    # G as NLVL scaled fp8 residual levels
    g_levels = []
    res = G.astype(np.float64)
    for i in range(NLVL):
        s = GS0 * GLS**i
        p8 = (res * s).astype(ml_dtypes.float8_e3m4)
        g_levels.append(p8)
        res = res - p8.astype(np.float64) / s
    gp = np.zeros((128, NT, NLVL, NB), dtype=ml_dtypes.float8_e3m4)
    for t in range(NT):
        for p in range(128):
            r = _rowmap(p, t)
            if r >= 0:
                for i in range(NLVL):
                    gp[p, t, i] = g_levels[i][r]

    # bf16 pos/pos^2 tables in the quadrant layout (pads are 0)
    pshift = 1.0 / (2.0 * L)
    pos = np.linspace(pshift, 1.0 - pshift, L).astype(np.float64)
    posq = np.zeros((100, 2 * (BLKA + BLKB)), dtype=np.float64)
    for q in range(4):
        for r_ in range(PER):
            p = 32 * q + r_
            posq[p, 0:BLKA] = pos[q * BLKA : (q + 1) * BLKA]
            b0 = HALF_A_ROWS + q * BLKB
            b1 = min(HALF_A_ROWS + (q + 1) * BLKB, L)
            nn = b1 - b0
            if nn > 0:
                posq[p, BLKA : BLKA + nn] = pos[b0:b1]
    p2 = BLKA + BLKB
    posq[:, p2 : 2 * p2] = posq[:, 0:p2] ** 2
    posb = posq.astype(ml_dtypes.bfloat16)

    # W^T/q^T tiles: wt[p, et, d] = 16*W_enc[d, et*128+p] (e4m3 subnormal
    # floor; the 1/16 folds into the exp scale); qt[p, et, b] = q[b, et*128+p]
    wt = np.ascontiguousarray(
        (W_enc.T * 16.0).reshape(4, 128, D).transpose(1, 0, 2)
    ).astype(ml_dtypes.float8_e4m3)
    misc = np.zeros((100, 80), dtype=np.float32)
    misc[:PER, 0:16] = np.tile(basis_mu, (PER, 1))
    misc[:PER, 16:32] = np.tile(basis_sigma**2, (PER, 1))
    misc[:16, 32:48] = np.eye(16, dtype=np.float32)
    for s in range(3):
        misc[:PER, 48 + s * PER : 48 + (s + 1) * PER] = np.eye(PER, dtype=np.float32)
    for p in range(100):
        if p % 32 < PER:
            misc[p, 64 + (p % 32)] = 1.0

    in_maps = []
    for c in range(NCORES):
        sl = slice(c * PER, (c + 1) * PER)
        qc = query[sl, 0, :]
        qt = np.ascontiguousarray(
            qc.T.reshape(4, 128, PER).transpose(1, 0, 2)
        ).astype(ml_dtypes.float8_e4m3)
        kta, ktb = _pack_ktstream(keys[sl])
        in_maps.append(
            {
                "ktpa": kta,
                "ktpb": ktb,
                "vp": _pack_vstream(values[sl]),
                "wt": wt,
                "qt": qt,
                "gp": gp,
                "posb": posb,
                "misc": misc,
            }
        )
    return in_maps


def kernel(query, keys, values, mask, W_enc, G, basis_mu, basis_sigma, **_kw):
    nc = _get_nc()
    in_maps = make_in_maps(query, keys, values, W_enc, G, basis_mu, basis_sigma)
    res = run_bass_kernel_spmd(nc, in_maps, core_ids=list(range(NCORES))).results
    out = np.stack([np.asarray(res[c]["out"]) for c in range(NCORES)])  # (8, PER, D)
    return out.reshape(B, 1, D).astype(np.float32)
